# revision 9
# baseline (speedup 1.0000x reference)
"""Trainium2 Bass kernel for nn_Composition_69372311765137.

result_means = quat_rotate(rotors[idx], means) + trans[idx]
result_quats = quat_mul(rotors[idx], quats)

Strategy (pure data parallel over 8 NeuronCores):
 - Shard means/quats/indices along the point axis across the 8 cores; pad each
   shard to 500,736 points (128 partitions x 489 pts x 8 tiles).
 - The tiny (64,3)/(64,4) trans/rotors tables are baked into the program as
   per-instruction scalar immediates (program is built after inputs are known),
   so there is no table DMA at all.
 - Gather = 64-object mask-accumulate on the Vector engine using a custom
   fused DVE op handling TWO objects per instruction:
       acc' = acc + (d==0)*v_k + (d==1)*v_{k+1}
   over a running down-shifted fp32 index d = idx - 2p.
 - Quaternion math runs on interleaved strided fp32 views (no unpack/pack).
"""

import numpy as np

N_CORES = 8
M_TOTAL = 4_000_000
M_CORE = M_TOTAL // N_CORES          # 500_000
J = 489                              # points per partition per tile
TILES = 8
M_PAD = 128 * J * TILES              # 500_736
ROWS = 128 * TILES
N_OBJ = 64

GSEL2 = None
GSEL2I = None


def _register_custom_ops():
    """Register the fused pair-gather DVE ops (idempotent)."""
    global GSEL2, GSEL2I
    if GSEL2 is not None:
        return
    from concourse.dve_spec import (Spec, Src0, Src1, C1, C2, Zero, One, eq,
                                    lower)
    from concourse.dve_ops import DveOp, OPS, get_dve_sub_opcode, has_src1
    from concourse.dve_uop import DveOpSpec

    import concourse.dve_ops as dve_ops_mod

    def _mk(name, body, ref):
        spec = Spec(body=body, reference=ref)
        # Register the new op name in the module-level tables first (the
        # sub-opcode mapping is built at import time from OPS).
        if name not in dve_ops_mod._SUB_OPCODE_FOR_NAME:
            dve_ops_mod._SUB_OPCODE_FOR_NAME[name] = (
                dve_ops_mod._CUSTOM_DVE_ROW_BASE + len(OPS))
            assert dve_ops_mod._SUB_OPCODE_FOR_NAME[name] < 0x20
        dve_ops_mod.CUSTOM_DVE_SPECS[name] = spec
        shas = {}
        for ver in ("v3",):
            uops = lower(spec, ver=ver)
            shas[ver] = DveOpSpec(
                name=name, opcode=get_dve_sub_opcode(name), uops=uops,
                rd1_en=has_src1(spec),
            ).sha(ver)
        op = DveOp(name, spec, subdim=False, uops_sha=shas)
        OPS.append(op)
        return op

    def _ref2(in0, in1, s0, s1, imm2):
        return (in1 + (in0 == 0) * s1 + (in0 == 1) * imm2).astype(np.float32)

    def _ref2i(in0, in1, s0, s1, imm2):
        return ((in0 == 0) * s1 + (in0 == 1) * imm2).astype(np.float32)

    GSEL2 = _mk("GSEL2", Src1 + eq(Src0, Zero) * C1 + eq(Src0, One) * C2, _ref2)
    GSEL2I = _mk("GSEL2I", eq(Src0, Zero) * C1 + eq(Src0, One) * C2, _ref2i)


_PROGRAM_CACHE = {}


def _build_program(table):
    """table: np.ndarray [64, 7] fp32 cols = [tx,ty,tz,rw,rx,ry,rz]."""
    import concourse.bacc as bacc
    import concourse.mybir as mybir
    import concourse.tile as tile

    _register_custom_ops()

    f32 = mybir.dt.float32
    i32 = mybir.dt.int32
    Alu = mybir.AluOpType

    nc = bacc.Bacc("TRN2", target_bir_lowering=False, debug=False,
                   num_devices=N_CORES)

    d_means = nc.dram_tensor("means", [ROWS, 3 * J], f32, kind="ExternalInput")
    d_quats = nc.dram_tensor("quats", [ROWS, 4 * J], f32, kind="ExternalInput")
    d_idx = nc.dram_tensor("indices", [ROWS, J], i32, kind="ExternalInput")
    d_om = nc.dram_tensor("out_means", [ROWS, 3 * J], f32, kind="ExternalOutput")
    d_oq = nc.dram_tensor("out_quats", [ROWS, 4 * J], f32, kind="ExternalOutput")

    T = [[float(table[k, c]) for c in range(7)] for k in range(N_OBJ)]

    V = nc.vector
    G = nc.gpsimd

    with tile.TileContext(nc) as tc:
        with tc.tile_pool(name="io", bufs=2) as iop, \
             tc.tile_pool(name="wk", bufs=3) as wp, \
             tc.tile_pool(name="ac", bufs=2) as ap_:
            for t in range(TILES):
                rs = slice(128 * t, 128 * t + 128)
                vm = iop.tile([128, 3 * J], f32, tag="vm")
                vq = iop.tile([128, 4 * J], f32, tag="vq")
                vi = iop.tile([128, J], i32, tag="vi")
                nc.sync.dma_start(out=vm[:], in_=d_means.ap()[rs, :])
                nc.sync.dma_start(out=vq[:], in_=d_quats.ap()[rs, :])
                nc.sync.dma_start(out=vi[:], in_=d_idx.ap()[rs, :])

                idxd = wp.tile([128, J], f32, tag="idxd")
                G.tensor_copy(out=idxd[:], in_=vi[:])   # i32 -> f32 cast

                accs = [ap_.tile([128, J], f32, tag=f"acc{c}", name=f"acc{c}") for c in range(7)]

                # ---- gather: 32 pairs of objects on DVE ----
                for p in range(32):
                    if p > 0:
                        V.tensor_scalar_sub(idxd[:], idxd[:], 2.0)
                    op = GSEL2I if p == 0 else GSEL2
                    for c in range(7):
                        kw = dict(out=accs[c][:], in0=idxd[:],
                                  s1=T[2 * p][c], imm2=T[2 * p + 1][c])
                        if p > 0:
                            kw["in1"] = accs[c][:]
                        V._custom_dve(op, **kw)

                tx, ty, tz, rw, rx, ry, rz = [a[:] for a in accs]

                vmf = vm[:].rearrange("p (j c) -> p j c", c=3)
                vqf = vq[:].rearrange("p (j c) -> p j c", c=4)
                vx, vy, vz = vmf[:, :, 0], vmf[:, :, 1], vmf[:, :, 2]
                qw, qx, qy, qz = (vqf[:, :, 0], vqf[:, :, 1],
                                  vqf[:, :, 2], vqf[:, :, 3])

                om = iop.tile([128, 3 * J], f32, tag="om")
                oq = iop.tile([128, 4 * J], f32, tag="oq")
                omf = om[:].rearrange("p (j c) -> p j c", c=3)
                oqf = oq[:].rearrange("p (j c) -> p j c", c=4)

                def tmp(tag):
                    return wp.tile([128, J], f32, tag=tag, name=tag)[:]

                # ---- t1 = cross(r_xyz, v) ----
                t1 = []
                for ci, (a, b, u, w) in enumerate(((ry, rz, vz, vy),
                                                   (rz, rx, vx, vz),
                                                   (rx, ry, vy, vx))):
                    m1 = tmp("m1"); m2 = tmp("m2")
                    V.tensor_mul(out=m1, in0=a, in1=u)
                    V.tensor_mul(out=m2, in0=b, in1=w)
                    t1c = tmp(f"t1{ci}")
                    V.tensor_sub(out=t1c, in0=m1, in1=m2)
                    t1.append(t1c)
                t1x, t1y, t1z = t1

                # ---- v' = v + 2w*t1 + 2*cross(r_xyz, t1) + t ----
                crs2 = ((ry, rz, t1z, t1y), (rz, rx, t1x, t1z),
                        (rx, ry, t1y, t1x))
                for c, ((a, b, u, w), vc, tc_, oc) in enumerate(
                        zip(crs2, (vx, vy, vz), (tx, ty, tz),
                            (omf[:, :, 0], omf[:, :, 1], omf[:, :, 2]))):
                    s = tmp("s")
                    V.scalar_tensor_tensor(out=s, in0=t1[c], scalar=2.0,
                                           in1=rw, op0=Alu.mult, op1=Alu.mult)
                    uacc = tmp("u")
                    V.tensor_add(out=uacc, in0=vc, in1=s)
                    m1 = tmp("m1"); m2 = tmp("m2")
                    V.tensor_mul(out=m1, in0=a, in1=u)
                    V.tensor_mul(out=m2, in0=b, in1=w)
                    c2 = tmp("c2")
                    V.tensor_sub(out=c2, in0=m1, in1=m2)
                    wv = tmp("w")
                    V.scalar_tensor_tensor(out=wv, in0=c2, scalar=2.0,
                                           in1=uacc, op0=Alu.mult, op1=Alu.add)
                    V.tensor_add(out=oc, in0=wv, in1=tc_)

                # ---- quat_mul(r, q) ----
                QM = (
                    ((rw, qw, 1), (rx, qx, -1), (ry, qy, -1), (rz, qz, -1)),
                    ((rw, qx, 1), (rx, qw, 1), (ry, qz, 1), (rz, qy, -1)),
                    ((rw, qy, 1), (rx, qz, -1), (ry, qw, 1), (rz, qx, 1)),
                    ((rw, qz, 1), (rx, qy, 1), (ry, qx, -1), (rz, qw, 1)),
                )
                for c in range(4):
                    oc = oqf[:, :, c]
                    acc = None
                    for (a, b, sgn) in QM[c]:
                        m = tmp("qm")
                        V.tensor_mul(out=m, in0=a, in1=b)
                        if acc is None:
                            acc = m
                        else:
                            nxt = oc if (a, b, sgn) == QM[c][3] else tmp("qa")
                            V.tensor_tensor(out=nxt, in0=acc, in1=m,
                                            op=Alu.add if sgn > 0
                                            else Alu.subtract)
                            acc = nxt

                nc.sync.dma_start(out=d_om.ap()[rs, :], in_=om[:])
                nc.sync.dma_start(out=d_oq.ap()[rs, :], in_=oq[:])

    nc.compile()
    return nc


def _prep_core_inputs(means, quats, indices):
    in_maps = []
    for i in range(N_CORES):
        s = slice(i * M_CORE, (i + 1) * M_CORE)
        pad = M_PAD - M_CORE
        m = np.concatenate([means[s], np.zeros((pad, 3), np.float32)])
        q = np.concatenate([quats[s], np.zeros((pad, 4), np.float32)])
        ix = np.concatenate([indices[s], np.zeros(pad, np.int32)])
        in_maps.append({
            "means": np.ascontiguousarray(m.reshape(ROWS, 3 * J)),
            "quats": np.ascontiguousarray(q.reshape(ROWS, 4 * J)),
            "indices": np.ascontiguousarray(ix.reshape(ROWS, J)),
        })
    return in_maps


def _run(trans, rotors, means, quats, indices, trace=False):
    from concourse.bass_utils import run_bass_kernel_spmd

    trans = np.asarray(trans, np.float32)
    rotors = np.asarray(rotors, np.float32)
    means = np.asarray(means, np.float32)
    quats = np.asarray(quats, np.float32)
    indices = np.asarray(indices, np.int32)

    table = np.concatenate([trans, rotors], axis=1)  # [64, 7]
    key = table.tobytes()
    nc = _PROGRAM_CACHE.get(key)
    if nc is None:
        nc = _build_program(table)
        _PROGRAM_CACHE[key] = nc

    in_maps = _prep_core_inputs(means, quats, indices)
    res = run_bass_kernel_spmd(nc, in_maps, core_ids=list(range(N_CORES)),
                               trace=trace)

    oms, oqs = [], []
    for i in range(N_CORES):
        om = res.results[i]["out_means"].reshape(M_PAD, 3)[:M_CORE]
        oq = res.results[i]["out_quats"].reshape(M_PAD, 4)[:M_CORE]
        oms.append(om)
        oqs.append(oq)
    out_means = np.concatenate(oms).astype(np.float32)
    out_quats = np.concatenate(oqs).astype(np.float32)
    return (out_means, out_quats), res


def kernel(trans, rotors, means, quats, indices):
    (out_means, out_quats), _ = _run(trans, rotors, means, quats, indices)
    return out_means, out_quats


def _timed_run(trans, rotors, means, quats, indices, reps=8, n_time=5):
    """Measure on-device exec time by chaining `reps` executions of the bass
    program inside one jitted call (call i+1 consumes call i's outputs as its
    donated output buffers, which defeats CSE and serializes the calls).
    Returns (outputs, est_exec_seconds_per_rep)."""
    import jax
    import time as _time
    from jax.sharding import Mesh, PartitionSpec
    from jax.experimental.shard_map import shard_map
    import concourse.mybir as mybir
    from concourse import bass2jax

    trans = np.asarray(trans, np.float32)
    rotors = np.asarray(rotors, np.float32)
    means = np.asarray(means, np.float32)
    quats = np.asarray(quats, np.float32)
    indices = np.asarray(indices, np.int32)
    table = np.concatenate([trans, rotors], axis=1)
    key = table.tobytes()
    nc = _PROGRAM_CACHE.get(key)
    if nc is None:
        nc = _build_program(table)
        _PROGRAM_CACHE[key] = nc
    in_maps = _prep_core_inputs(means, quats, indices)

    bass2jax.install_neuronx_cc_hook()
    partition_name = (nc.partition_id_tensor.name
                      if nc.partition_id_tensor else None)
    in_names, out_names, out_avals, zero_outs = [], [], [], []
    for alloc in nc.m.functions[0].allocations:
        if not isinstance(alloc, mybir.MemoryLocationSet):
            continue
        name = alloc.memorylocations[0].name
        if alloc.kind == "ExternalInput":
            if name != partition_name:
                in_names.append(name)
        elif alloc.kind == "ExternalOutput":
            out_names.append(name)
            shape = tuple(alloc.tensor_shape)
            dtype = mybir.dt.np(alloc.dtype)
            out_avals.append(jax.core.ShapedArray(shape, dtype))
            zero_outs.append(np.zeros(shape, dtype))
    n_params = len(in_names)
    n_outs = len(out_avals)
    all_in_names = list(in_names) + list(out_names)
    if partition_name is not None:
        all_in_names.append(partition_name)

    def _body(*args):
        operands = list(args)
        if partition_name is not None:
            operands.append(bass2jax.partition_id_tensor())
        return tuple(bass2jax._bass_exec_p.bind(
            *operands,
            out_avals=tuple(out_avals),
            in_names=tuple(all_in_names),
            out_names=tuple(out_names),
            lowering_input_output_aliases=(),
            sim_require_finite=True,
            sim_require_nnan=True,
            nc=nc,
        ))

    def _rep(*args):
        ins = args[:n_params]
        outs = tuple(args[n_params:])
        return _body(*ins, *outs)

    devices = jax.devices()[:N_CORES]
    mesh = Mesh(np.asarray(devices), ("core",))
    in_specs = (PartitionSpec("core"),) * (n_params + n_outs)
    out_specs = (PartitionSpec("core"),) * n_outs
    donate = tuple(range(n_params, n_params + n_outs))
    sharded = jax.jit(
        shard_map(_rep, mesh=mesh, in_specs=in_specs, out_specs=out_specs,
                  check_rep=False),
        donate_argnums=donate, keep_unused=True)

    per_core = [[np.asarray(m[nm]) for nm in in_names] for m in in_maps]
    concat_in = [np.concatenate([per_core[c][i] for c in range(N_CORES)])
                 for i in range(n_params)]
    concat_zeros = [np.zeros((N_CORES * z.shape[0], *z.shape[1:]), z.dtype)
                    for z in zero_outs]
    sh = jax.sharding.NamedSharding(mesh, PartitionSpec("core"))
    dev_in = [jax.device_put(a, sh) for a in concat_in]
    outs = [jax.device_put(a, sh) for a in concat_zeros]

    # Warmup (compiles the XLA wrapper once).
    outs = list(sharded(*dev_in, *outs))
    for o in outs:
        o.block_until_ready()

    times = []
    for it in range(n_time):
        t0 = _time.perf_counter()
        for _ in range(reps):
            outs = list(sharded(*dev_in, *outs))
        for o in outs:
            o.block_until_ready()
        times.append(_time.perf_counter() - t0)
    best = min(times)

    res = {}
    for i, nm in enumerate(out_names):
        res[nm] = np.asarray(outs[i]).reshape(N_CORES, *out_avals[i].shape)
    oms = [res["out_means"][c].reshape(M_PAD, 3)[:M_CORE]
           for c in range(N_CORES)]
    oqs = [res["out_quats"][c].reshape(M_PAD, 4)[:M_CORE]
           for c in range(N_CORES)]
    out_means = np.concatenate(oms).astype(np.float32)
    out_quats = np.concatenate(oqs).astype(np.float32)
    return (out_means, out_quats), best / reps, times
